# revision 25
# baseline (speedup 1.0000x reference)
"""KANLayer (in=128, out=128, num=5, k=3, batch=1024) on 8 trn2 NeuronCores.

Math: out[b,o] = sum_i mask*scale_base*silu(x[b,i])
              + sum_i mask*scale_sp*sum_j coef[(o,i),j]*B_j(x[b,i])
The reference grid is a uniform linspace broadcast to all rows, so the
Cox-de-Boor bases are cardinal cubic B-splines: B_j(x) = N3(v - j) with
v = (x - g0ext)/h.  N3 is evaluated in closed (truncated-power) form:
6*N3(v - j) = Delta^4[relu(v - n)^3] at n = j, so the whole basis bank
[128 i-lanes, 8 bases, batch] comes from one relu^3 over 12 taps plus
four shifted-slice subtracts (the 1/6 is folded into the spline weights
host-side).  The contraction over (i, j) and the silu base term are 9
accumulated 128x128x128 PE matmuls per core (bf16 in, f32 accumulate).

Valid for |x| <= g_ext_max (|x| <= 8.8 here); beyond that the Delta^4
cancellation noise grows cubically.  setup_inputs' randn never leaves
[-5, 5].

Sharding: batch 1024 -> 128 per core (independent; no collectives).

Execution: the Bass program is AOT-compiled once into a PJRT executable
(fast-dispatch, no per-call retrace) and dispatched on cores 0-7; falls
back to the stock run_bass_kernel_spmd path on any failure.
"""

import numpy as np

import concourse.bass as bass
import concourse.mybir as mybir
import concourse.tile as tile

AF = mybir.ActivationFunctionType
ALU = mybir.AluOpType
F32 = mybir.dt.float32
BF16 = mybir.dt.bfloat16

N_CORES = 8
BATCH = 1024
IN_DIM = 128
OUT_DIM = 128
NUM, KDEG = 5, 3
NB = NUM + KDEG          # 8 basis functions
NK = 1 + NB              # 9 matmul K-tiles (silu + 8 bases)
NT = NB + 4              # 12 truncated-power taps relu(v-n)^3, n = 0..11
BSH = BATCH // N_CORES   # 128 batch elems per core
SIZE = IN_DIM * OUT_DIM


def _bcast_mid(ap2d, n):
    """[128, F] AP -> [128, n, F] with zero-stride middle dim."""
    p, f = ap2d.shape
    return ap2d.rearrange("p (a b) -> p a b", a=1).broadcast_to([p, n, f])


def _flat(ap3d):
    """[128, a, b] AP -> [128, a*b]."""
    return ap3d.rearrange("p a b -> p (a b)")


MM_DT = BF16  # matmul operand dtype (weights, silu, basis bank)

# Live-tap window, set by prepare_inputs from the actual data range:
# tap n contributes only if max(v) > n, and relu is needed for tap n only
# if min(v) < n.  For randn x on this grid: v in [2.7, 8.3] -> L=9, N0=3.
TAP_L = NT
TAP_N0 = 0


def _emit_iter(nc, pool, psum, xs, wt, outT, inv_h, bias_v, split=False):
    """One full kernel pass: load, spline-basis bank, 9 matmuls, store.

    outT is a [OUT_DIM, BSH] dram AP (a per-iteration slice when unrolled
    pipelined).  split=False keeps the whole basis chain on the vector
    engine — same-engine deps are free and cross-engine hops cost ~1us,
    which is optimal when iterations serialize.  split=True interleaves
    vector and gpsimd per level for the double-buffered streaming case,
    where hops hide and the per-engine queue length is what matters.
    Every Delta level is one flat 2D op (in1 = same buffer shifted by BSH
    elements); 3D strided APs cost ~0.7us extra per op on DVE.
    """
    L = TAP_L  # L in [9, 12]
    h = 1.0 / inv_h
    g0ext = -bias_v * h
    eng2 = nc.gpsimd if split else nc.vector
    X = pool.tile([128, BSH], F32, tag="X")
    nc.sync.dma_start(X[:], xs[:])
    WT = pool.tile([128, NK, OUT_DIM], MM_DT, tag="WT")
    nc.sync.dma_start(WT[:].rearrange("p a b -> p (a b)"), wt[:])

    S = pool.tile([128, BSH], MM_DT, tag="S")  # silu(x), matmul K-tile 0
    nc.scalar.activation(S[:], X[:], AF.Silu)

    # negated knot positions -t_n = -(g0ext + n*h) in x-units, so taps
    # come straight off X with no affine (the h^-3 rescale is folded into
    # the spline weights host-side).  Loop-constant; double-buffered so
    # the next iteration's writer never waits on this one's reader.
    ICW = pool.tile([128, L, BSH], F32, tag="ICW", bufs=2)
    nc.gpsimd.iota(
        ICW[:], pattern=[[-1, L], [0, BSH]], base=0, channel_multiplier=0,
        allow_small_or_imprecise_dtypes=True,
    )
    nc.gpsimd.tensor_scalar(ICW[:], ICW[:], h, -g0ext, ALU.mult, ALU.add)

    D = pool.tile([128, L, BSH], F32, tag="D")      # x - t_n
    nc.vector.tensor_tensor(D[:], ICW[:], _bcast_mid(X[:], L), ALU.add)
    # relu on all taps (identity on the always-positive low taps)
    R = pool.tile([128, L, BSH], F32, tag="R")      # relu(x - t_n)
    nc.vector.tensor_scalar(_flat(R[:]), _flat(D[:]), 0.0, None, ALU.max)
    R2 = pool.tile([128, L, BSH], F32, tag="R2")
    eng2.tensor_tensor(_flat(R2[:]), _flat(R[:]), _flat(R[:]), ALU.mult)
    R3 = pool.tile([128, L + 1, BSH], F32, tag="R3")
    R3f = _flat(R3[:])
    nc.vector.tensor_tensor(R3f[:, : L * BSH], _flat(R2[:]), _flat(R[:]),
                            ALU.mult)
    eng2.memset(R3f[:, L * BSH :], 0.0)

    # BB[j] = Delta^4 R3 | j == 6 * B_j(v).  Rows >= L of each level are
    # identically zero (taps above the data range), kept as memset tail
    # rows so every level stays one op.
    D1 = pool.tile([128, L + 1, BSH], F32, tag="D1")
    D1f = _flat(D1[:])
    eng2.tensor_tensor(D1f[:, : L * BSH], R3f[:, BSH:],
                       R3f[:, : L * BSH], ALU.subtract)
    eng2.memset(D1f[:, L * BSH :], 0.0)
    D2 = pool.tile([128, L + 1, BSH], F32, tag="D2")
    D2f = _flat(D2[:])
    nc.vector.tensor_tensor(D2f[:, : L * BSH], D1f[:, BSH:],
                            D1f[:, : L * BSH], ALU.subtract)
    nc.vector.memset(D2f[:, L * BSH :], 0.0)
    D3 = pool.tile([128, NB + 1, BSH], F32, tag="D3")
    D3f = _flat(D3[:])
    eng2.tensor_tensor(D3f[:], D2f[:, BSH : (NB + 2) * BSH],
                       D2f[:, : (NB + 1) * BSH], ALU.subtract)
    BB = pool.tile([128, NB, BSH], MM_DT, tag="BB")
    nc.vector.tensor_tensor(_flat(BB[:]), D3f[:, BSH:], D3f[:, : NB * BSH],
                            ALU.subtract)

    # out^T[o,b] = sum_k WT[:,k,:]^T @ rhs_k, K = 9*128
    PS = psum.tile([OUT_DIM, BSH], F32, tag="PS")
    for k in range(NK):
        rhs = S[:] if k == 0 else BB[:, k - 1, :]
        nc.tensor.matmul(
            PS[:], WT[:, k, :], rhs, start=(k == 0), stop=(k == NK - 1)
        )
    O = pool.tile([OUT_DIM, BSH], F32, tag="O")
    nc.scalar.copy(O[:], PS[:])
    nc.sync.dma_start(outT[:, :], O[:])


def build_program(
    inv_h: float, bias_v: float, iters: int = 1, pipelined: bool = False
):
    """One SPMD NeuronCore program; per-core inputs differ only in data.

    iters > 1 unrolls the full kernel back-to-back inside one NEFF — used
    to measure per-iteration HW execution time without a profiler.
    pipelined=True double-buffers tiles and gives each iteration its own
    output slice (streaming steady state); False reuses single buffers,
    serializing iterations (per-pass latency).
    """
    nc = bass.Bass()
    xs = nc.declare_dram_parameter("xs", [IN_DIM, BSH], F32, isOutput=False)
    # weights pre-transposed host-side to [i, k*o] so the load is one
    # contiguous-per-partition DMA (the [k*i, o] layout needs a 1152-
    # descriptor gather, ~3us of DMA-queue time per pass)
    wt = nc.declare_dram_parameter(
        "wt", [128, NK * OUT_DIM], MM_DT, isOutput=False
    )
    n_out = iters if pipelined else 1
    outT = nc.declare_dram_parameter(
        "outT", [OUT_DIM, n_out * BSH], F32, isOutput=True
    )

    with tile.TileContext(nc) as tc:
        with (
            tc.tile_pool(name="pool", bufs=2 if pipelined else 1) as pool,
            tc.tile_pool(
                name="psum", bufs=2 if pipelined else 1,
                space=bass.MemorySpace.PSUM,
            ) as psum,
        ):
            for it in range(iters):
                o = outT[:, it * BSH : (it + 1) * BSH] if pipelined else outT[:]
                _emit_iter(nc, pool, psum, xs, wt, o, inv_h, bias_v,
                           split=pipelined)

    return nc


def _legalize_waits(nc):
    """Walrus codegen allows only one semaphore wait per compute/DMA
    instruction; move extra waits onto inserted same-engine NoOps."""
    for blk in nc.m.functions[0].blocks:
        out = []
        for ins in blk.instructions:
            si = ins.sync_info
            if si is not None and len(si.on_wait) > 1:
                waits = list(si.on_wait)
                for i, w in enumerate(waits[:-1]):
                    nop = mybir.InstNoOp(
                        name=f"{ins.name}-lw{i}", engine=ins.engine, ins=[], outs=[]
                    )
                    nop.sync_info = mybir.SyncInfo(on_wait=[w], on_update=[])
                    out.append(nop)
                ins.sync_info = mybir.SyncInfo(
                    on_wait=[waits[-1]], on_update=list(si.on_update)
                )
            out.append(ins)
        blk.instructions = out
    return nc


def prepare_inputs(x, grid, coef, scale_base, scale_sp, mask):
    global TAP_L, TAP_N0
    x = np.ascontiguousarray(x, dtype=np.float32)
    grid = np.asarray(grid, dtype=np.float32)
    coef = np.asarray(coef, dtype=np.float32)
    g = grid[0].astype(np.float64)
    h = (g[-1] - g[0]) / (len(g) - 1)
    g0ext = g[0] - KDEG * h
    inv_h = 1.0 / h
    bias_v = -g0ext * inv_h

    # live-tap window from the actual data range (v = x*inv_h + bias_v):
    # taps >= L are identically zero, taps < N0 never need the relu
    vmin = float(x.min()) * inv_h + bias_v
    vmax = float(x.max()) * inv_h + bias_v
    TAP_L = int(min(max(np.floor(vmax) + 1, 9), NT))
    TAP_N0 = int(max(min(np.floor(vmin) + 1, TAP_L), 0))

    import ml_dtypes

    sbm = (np.asarray(scale_base) * np.asarray(mask)).astype(np.float32)
    sspm = (np.asarray(scale_sp) * np.asarray(mask)).astype(np.float32)
    wt = np.empty((NK * 128, OUT_DIM), np.float32)
    wt[0:128] = sbm.reshape(OUT_DIM, IN_DIM).T
    # fold the 1/(6 h^3) of the x-unit truncated-power form into the
    # spline weights
    spl = float(inv_h**3 / 6.0)
    for j in range(NB):
        wt[(j + 1) * 128 : (j + 2) * 128] = (
            (sspm * coef[:, j] * spl).reshape(OUT_DIM, IN_DIM).T
        )
    # [k*i, o] -> [i, k*o] so each partition's weights are contiguous
    wt = np.ascontiguousarray(
        wt.reshape(NK, IN_DIM, OUT_DIM).transpose(1, 0, 2).reshape(
            IN_DIM, NK * OUT_DIM
        )
    ).astype(mybir.dt.np(MM_DT))
    xT = np.ascontiguousarray(x.T)  # [i, b]
    in_maps = [
        {
            "xs": np.ascontiguousarray(xT[:, c * BSH : (c + 1) * BSH]),
            "wt": wt,
        }
        for c in range(N_CORES)
    ]
    return in_maps, float(inv_h), float(bias_v)


class Runner:
    """AOT-compiled fast-dispatch executor for a Bass program on N cores.

    Compiles once (jit trace + NEFF build happen here, not per call);
    subsequent __call__s hit JAX's C++ fast path — per-call cost is the
    axon dispatch plus device execution only.
    """

    def __init__(self, nc, n_cores: int = N_CORES):
        import jax
        from jax.sharding import Mesh, NamedSharding, PartitionSpec

        from concourse import bass2jax
        from concourse.bass2jax import (
            _bass_exec_p,
            fast_dispatch_compile,
            install_neuronx_cc_hook,
        )

        try:
            from jax.experimental.shard_map import shard_map
        except ImportError:  # newer jax
            from jax import shard_map

        install_neuronx_cc_hook()
        self.jax = jax
        self.n_cores = n_cores
        part_name = nc.partition_id_tensor.name if nc.partition_id_tensor else None
        assert nc.dbg_addr is None

        in_names, in_shapes, out_names, out_avals = [], [], [], []
        for alloc in nc.m.functions[0].allocations:
            if not isinstance(alloc, mybir.MemoryLocationSet):
                continue
            name = alloc.memorylocations[0].name
            if alloc.kind == "ExternalInput":
                if name != part_name:
                    in_names.append(name)
                    in_shapes.append(
                        (tuple(alloc.tensor_shape), mybir.dt.np(alloc.dtype))
                    )
            elif alloc.kind == "ExternalOutput":
                out_names.append(name)
                out_avals.append(
                    jax.core.ShapedArray(
                        tuple(alloc.tensor_shape), mybir.dt.np(alloc.dtype)
                    )
                )
        self.in_names = in_names
        self.out_names = out_names
        self.out_avals = out_avals
        # The kernel writes every element of its outputs, so they are not
        # passed as (donated zero) operands — results are fresh buffers.
        all_in_names = list(in_names)
        if part_name is not None:
            all_in_names.append(part_name)

        def _body(*args):
            operands = list(args)
            if part_name is not None:
                operands.append(bass2jax.partition_id_tensor())
            outs = _bass_exec_p.bind(
                *operands,
                out_avals=tuple(out_avals),
                in_names=tuple(all_in_names),
                out_names=tuple(out_names),
                lowering_input_output_aliases=(),
                sim_require_finite=True,
                sim_require_nnan=True,
                nc=nc,
            )
            return tuple(outs)

        devices = jax.devices()[:n_cores]
        self.mesh = Mesh(np.asarray(devices), ("core",))
        self.sharding = NamedSharding(self.mesh, PartitionSpec("core"))
        in_specs = (PartitionSpec("core"),) * len(in_names)
        out_specs = (PartitionSpec("core"),) * len(out_names)
        jitted = jax.jit(
            shard_map(
                _body,
                mesh=self.mesh,
                in_specs=in_specs,
                out_specs=out_specs,
                check_rep=False,
            ),
            keep_unused=True,
        )

        def compile_fn():
            abstract = [
                jax.ShapeDtypeStruct((n_cores * s[0], *s[1:]), d)
                for (s, d) in in_shapes
            ]
            return jitted.lower(*abstract).compile()

        self.compiled = fast_dispatch_compile(compile_fn)

    def stage(self, in_maps):
        """Concat per-core inputs on axis 0 and put on device (committed)."""
        concat = [
            np.concatenate(
                [np.asarray(in_maps[c][nm]) for c in range(self.n_cores)], axis=0
            )
            for nm in self.in_names
        ]
        args = [self.jax.device_put(a, self.sharding) for a in concat]
        self.jax.block_until_ready(args)
        return args

    def __call__(self, args):
        return self.compiled(*args)

    def fetch_np(self, outs):
        """outs -> list of per-core np arrays for output 0."""
        arr = np.asarray(outs[0])
        s = self.out_avals[0].shape
        return arr.reshape(self.n_cores, *s)


def _assemble(per_core_outT):
    """per-core outT [OUT_DIM, BSH] -> full [BATCH, OUT_DIM]."""
    return np.ascontiguousarray(
        np.concatenate([o.T for o in per_core_outT], axis=0), dtype=np.float32
    )


def run(inputs: dict, trace: bool = False, **spmd_kwargs):
    """Stock-path execution (kept for debugging / fallback)."""
    from concourse.bass_utils import run_bass_kernel_spmd

    in_maps, inv_h, bias_v = prepare_inputs(**inputs)
    nc = _legalize_waits(build_program(inv_h, bias_v))
    res = run_bass_kernel_spmd(
        nc, in_maps, list(range(N_CORES)), trace=trace, **spmd_kwargs
    )
    out = _assemble([np.asarray(res.results[c]["outT"]) for c in range(N_CORES)])
    return out, res


def kernel(**inputs) -> np.ndarray:
    assert inputs["x"].shape == (BATCH, IN_DIM)
    in_maps, inv_h, bias_v = prepare_inputs(**inputs)
    nc = _legalize_waits(build_program(inv_h, bias_v))
    try:
        runner = Runner(nc)
        outs = runner(runner.stage(in_maps))
        return _assemble(list(runner.fetch_np(outs)))
    except Exception:
        from concourse.bass_utils import run_bass_kernel_spmd

        res = run_bass_kernel_spmd(nc, in_maps, list(range(N_CORES)))
        return _assemble(
            [np.asarray(res.results[c]["outT"]) for c in range(N_CORES)]
        )


# revision 27
# speedup vs baseline: 1.0756x; 1.0756x over previous
"""KANLayer (in=128, out=128, num=5, k=3, batch=1024) on 8 trn2 NeuronCores.

Math: out[b,o] = sum_i mask*scale_base*silu(x[b,i])
              + sum_i mask*scale_sp*sum_j coef[(o,i),j]*B_j(x[b,i])
The reference grid is a uniform linspace broadcast to all rows, so the
Cox-de-Boor bases are cardinal cubic B-splines: B_j(x) = N3(v - j) with
v = (x - g0ext)/h.  N3 is evaluated in closed (truncated-power) form:
6*N3(v - j) = Delta^4[relu(v - n)^3] at n = j, so the whole basis bank
[128 i-lanes, 8 bases, batch] comes from one relu^3 over 12 taps plus
four shifted-slice subtracts (the 1/6 is folded into the spline weights
host-side).  The contraction over (i, j) and the silu base term are 9
accumulated 128x128x128 PE matmuls per core (bf16 in, f32 accumulate).

Valid for |x| <= g_ext_max (|x| <= 8.8 here); beyond that the Delta^4
cancellation noise grows cubically.  setup_inputs' randn never leaves
[-5, 5].

Sharding: batch 1024 -> 128 per core (independent; no collectives).

Execution: the Bass program is AOT-compiled once into a PJRT executable
(fast-dispatch, no per-call retrace) and dispatched on cores 0-7; falls
back to the stock run_bass_kernel_spmd path on any failure.
"""

import numpy as np

import concourse.bass as bass
import concourse.mybir as mybir
import concourse.tile as tile

AF = mybir.ActivationFunctionType
ALU = mybir.AluOpType
F32 = mybir.dt.float32
BF16 = mybir.dt.bfloat16

N_CORES = 8
BATCH = 1024
IN_DIM = 128
OUT_DIM = 128
NUM, KDEG = 5, 3
NB = NUM + KDEG          # 8 basis functions
NK = 1 + NB              # 9 matmul K-tiles (silu + 8 bases)
NT = NB + 4              # 12 truncated-power taps relu(v-n)^3, n = 0..11
BSH = BATCH // N_CORES   # 128 batch elems per core
SIZE = IN_DIM * OUT_DIM


def _bcast_mid(ap2d, n):
    """[128, F] AP -> [128, n, F] with zero-stride middle dim."""
    p, f = ap2d.shape
    return ap2d.rearrange("p (a b) -> p a b", a=1).broadcast_to([p, n, f])


def _flat(ap3d):
    """[128, a, b] AP -> [128, a*b]."""
    return ap3d.rearrange("p a b -> p (a b)")


MM_DT = BF16  # matmul operand dtype (weights, silu, basis bank)

# Live-tap window, set by prepare_inputs from the actual data range:
# tap n contributes only if max(v) > n, and relu is needed for tap n only
# if min(v) < n.  For randn x on this grid: v in [2.7, 8.3] -> L=9, N0=3.
TAP_L = NT
TAP_N0 = 0


def _emit_iter(nc, pool, psum, xs, wt, outT, inv_h, bias_v, split=False):
    """One full kernel pass: load, spline-basis bank, 9 matmuls, store.

    outT is a [OUT_DIM, BSH] dram AP (a per-iteration slice when unrolled
    pipelined).  split=False keeps the whole basis chain on the vector
    engine — same-engine deps are free and cross-engine hops cost ~1us,
    which is optimal when iterations serialize.  split=True interleaves
    vector and gpsimd per level for the double-buffered streaming case,
    where hops hide and the per-engine queue length is what matters.
    Every Delta level is one flat 2D op (in1 = same buffer shifted by BSH
    elements); 3D strided APs cost ~0.7us extra per op on DVE.
    """
    L = TAP_L  # L in [9, 12]
    eng2 = nc.gpsimd if split else nc.vector
    X = pool.tile([128, BSH], F32, tag="X")
    nc.sync.dma_start(X[:], xs[:])
    WT = pool.tile([128, NK, OUT_DIM], MM_DT, tag="WT")
    nc.sync.dma_start(WT[:].rearrange("p a b -> p (a b)"), wt[:])

    S = pool.tile([128, BSH], MM_DT, tag="S")  # silu(x), matmul K-tile 0
    nc.scalar.activation(S[:], X[:], AF.Silu)

    # negated tap offsets -n (loop-constant; double-buffered so the next
    # iteration's iota never waits on this one's reader)
    ICW = pool.tile([128, L, BSH], F32, tag="ICW", bufs=2)
    nc.gpsimd.iota(
        ICW[:], pattern=[[-1, L], [0, BSH]], base=0, channel_multiplier=0,
        allow_small_or_imprecise_dtypes=True,
    )

    V = pool.tile([128, BSH], F32, tag="V")    # v = x/h - g0ext/h
    nc.vector.tensor_scalar(V[:], X[:], inv_h, bias_v, ALU.mult, ALU.add)
    D = pool.tile([128, L, BSH], F32, tag="D")      # v - n
    nc.vector.tensor_tensor(D[:], ICW[:], _bcast_mid(V[:], L), ALU.add)
    # relu on all taps (identity on the always-positive low taps)
    R = pool.tile([128, L, BSH], F32, tag="R")      # relu(v - n)
    nc.vector.tensor_scalar(_flat(R[:]), _flat(D[:]), 0.0, None, ALU.max)
    R2 = pool.tile([128, L, BSH], F32, tag="R2")
    eng2.tensor_tensor(_flat(R2[:]), _flat(R[:]), _flat(R[:]), ALU.mult)
    R3 = pool.tile([128, L + 1, BSH], F32, tag="R3")
    R3f = _flat(R3[:])
    nc.vector.tensor_tensor(R3f[:, : L * BSH], _flat(R2[:]), _flat(R[:]),
                            ALU.mult)
    eng2.memset(R3f[:, L * BSH :], 0.0)

    # BB[j] = Delta^4 R3 | j == 6 * B_j(v).  Rows >= L of each level are
    # identically zero (taps above the data range), kept as memset tail
    # rows so every level stays one op.
    D1 = pool.tile([128, L + 1, BSH], F32, tag="D1")
    D1f = _flat(D1[:])
    eng2.tensor_tensor(D1f[:, : L * BSH], R3f[:, BSH:],
                       R3f[:, : L * BSH], ALU.subtract)
    eng2.memset(D1f[:, L * BSH :], 0.0)
    D2 = pool.tile([128, L + 1, BSH], F32, tag="D2")
    D2f = _flat(D2[:])
    nc.vector.tensor_tensor(D2f[:, : L * BSH], D1f[:, BSH:],
                            D1f[:, : L * BSH], ALU.subtract)
    nc.vector.memset(D2f[:, L * BSH :], 0.0)
    D3 = pool.tile([128, NB + 1, BSH], F32, tag="D3")
    D3f = _flat(D3[:])
    eng2.tensor_tensor(D3f[:], D2f[:, BSH : (NB + 2) * BSH],
                       D2f[:, : (NB + 1) * BSH], ALU.subtract)
    BB = pool.tile([128, NB, BSH], MM_DT, tag="BB")
    nc.vector.tensor_tensor(_flat(BB[:]), D3f[:, BSH:], D3f[:, : NB * BSH],
                            ALU.subtract)

    # out^T[o,b] = sum_k WT[:,k,:]^T @ rhs_k, K = 9*128
    PS = psum.tile([OUT_DIM, BSH], F32, tag="PS")
    for k in range(NK):
        rhs = S[:] if k == 0 else BB[:, k - 1, :]
        nc.tensor.matmul(
            PS[:], WT[:, k, :], rhs, start=(k == 0), stop=(k == NK - 1)
        )
    O = pool.tile([OUT_DIM, BSH], F32, tag="O")
    nc.scalar.copy(O[:], PS[:])
    nc.sync.dma_start(outT[:, :], O[:])


def build_program(
    inv_h: float, bias_v: float, iters: int = 1, pipelined: bool = False
):
    """One SPMD NeuronCore program; per-core inputs differ only in data.

    iters > 1 unrolls the full kernel back-to-back inside one NEFF — used
    to measure per-iteration HW execution time without a profiler.
    pipelined=True double-buffers tiles and gives each iteration its own
    output slice (streaming steady state); False reuses single buffers,
    serializing iterations (per-pass latency).
    """
    nc = bass.Bass()
    xs = nc.declare_dram_parameter("xs", [IN_DIM, BSH], F32, isOutput=False)
    # weights pre-transposed host-side to [i, k*o] so the load is one
    # contiguous-per-partition DMA (the [k*i, o] layout needs a 1152-
    # descriptor gather, ~3us of DMA-queue time per pass)
    wt = nc.declare_dram_parameter(
        "wt", [128, NK * OUT_DIM], MM_DT, isOutput=False
    )
    n_out = iters if pipelined else 1
    outT = nc.declare_dram_parameter(
        "outT", [OUT_DIM, n_out * BSH], F32, isOutput=True
    )

    with tile.TileContext(nc) as tc:
        with (
            tc.tile_pool(name="pool", bufs=2 if pipelined else 1) as pool,
            tc.tile_pool(
                name="psum", bufs=2 if pipelined else 1,
                space=bass.MemorySpace.PSUM,
            ) as psum,
        ):
            for it in range(iters):
                o = outT[:, it * BSH : (it + 1) * BSH] if pipelined else outT[:]
                _emit_iter(nc, pool, psum, xs, wt, o, inv_h, bias_v,
                           split=pipelined)

    return nc


def _legalize_waits(nc):
    """Walrus codegen allows only one semaphore wait per compute/DMA
    instruction; move extra waits onto inserted same-engine NoOps."""
    for blk in nc.m.functions[0].blocks:
        out = []
        for ins in blk.instructions:
            si = ins.sync_info
            if si is not None and len(si.on_wait) > 1:
                waits = list(si.on_wait)
                for i, w in enumerate(waits[:-1]):
                    nop = mybir.InstNoOp(
                        name=f"{ins.name}-lw{i}", engine=ins.engine, ins=[], outs=[]
                    )
                    nop.sync_info = mybir.SyncInfo(on_wait=[w], on_update=[])
                    out.append(nop)
                ins.sync_info = mybir.SyncInfo(
                    on_wait=[waits[-1]], on_update=list(si.on_update)
                )
            out.append(ins)
        blk.instructions = out
    return nc


def prepare_inputs(x, grid, coef, scale_base, scale_sp, mask):
    global TAP_L, TAP_N0
    x = np.ascontiguousarray(x, dtype=np.float32)
    grid = np.asarray(grid, dtype=np.float32)
    coef = np.asarray(coef, dtype=np.float32)
    g = grid[0].astype(np.float64)
    h = (g[-1] - g[0]) / (len(g) - 1)
    g0ext = g[0] - KDEG * h
    inv_h = 1.0 / h
    bias_v = -g0ext * inv_h

    # live-tap window from the actual data range (v = x*inv_h + bias_v):
    # taps >= L are identically zero, taps < N0 never need the relu
    vmin = float(x.min()) * inv_h + bias_v
    vmax = float(x.max()) * inv_h + bias_v
    TAP_L = int(min(max(np.floor(vmax) + 1, 9), NT))
    TAP_N0 = int(max(min(np.floor(vmin) + 1, TAP_L), 0))

    import ml_dtypes

    sbm = (np.asarray(scale_base) * np.asarray(mask)).astype(np.float32)
    sspm = (np.asarray(scale_sp) * np.asarray(mask)).astype(np.float32)
    wt = np.empty((NK * 128, OUT_DIM), np.float32)
    wt[0:128] = sbm.reshape(OUT_DIM, IN_DIM).T
    # fold the 1/6 of the truncated-power form into the spline weights
    for j in range(NB):
        wt[(j + 1) * 128 : (j + 2) * 128] = (
            (sspm * coef[:, j] / 6.0).reshape(OUT_DIM, IN_DIM).T
        )
    # [k*i, o] -> [i, k*o] so each partition's weights are contiguous
    wt = np.ascontiguousarray(
        wt.reshape(NK, IN_DIM, OUT_DIM).transpose(1, 0, 2).reshape(
            IN_DIM, NK * OUT_DIM
        )
    ).astype(mybir.dt.np(MM_DT))
    xT = np.ascontiguousarray(x.T)  # [i, b]
    in_maps = [
        {
            "xs": np.ascontiguousarray(xT[:, c * BSH : (c + 1) * BSH]),
            "wt": wt,
        }
        for c in range(N_CORES)
    ]
    return in_maps, float(inv_h), float(bias_v)


class Runner:
    """AOT-compiled fast-dispatch executor for a Bass program on N cores.

    Compiles once (jit trace + NEFF build happen here, not per call);
    subsequent __call__s hit JAX's C++ fast path — per-call cost is the
    axon dispatch plus device execution only.
    """

    def __init__(self, nc, n_cores: int = N_CORES):
        import jax
        from jax.sharding import Mesh, NamedSharding, PartitionSpec

        from concourse import bass2jax
        from concourse.bass2jax import (
            _bass_exec_p,
            fast_dispatch_compile,
            install_neuronx_cc_hook,
        )

        try:
            from jax.experimental.shard_map import shard_map
        except ImportError:  # newer jax
            from jax import shard_map

        install_neuronx_cc_hook()
        self.jax = jax
        self.n_cores = n_cores
        part_name = nc.partition_id_tensor.name if nc.partition_id_tensor else None
        assert nc.dbg_addr is None

        in_names, in_shapes, out_names, out_avals = [], [], [], []
        for alloc in nc.m.functions[0].allocations:
            if not isinstance(alloc, mybir.MemoryLocationSet):
                continue
            name = alloc.memorylocations[0].name
            if alloc.kind == "ExternalInput":
                if name != part_name:
                    in_names.append(name)
                    in_shapes.append(
                        (tuple(alloc.tensor_shape), mybir.dt.np(alloc.dtype))
                    )
            elif alloc.kind == "ExternalOutput":
                out_names.append(name)
                out_avals.append(
                    jax.core.ShapedArray(
                        tuple(alloc.tensor_shape), mybir.dt.np(alloc.dtype)
                    )
                )
        self.in_names = in_names
        self.out_names = out_names
        self.out_avals = out_avals
        # The kernel writes every element of its outputs, so they are not
        # passed as (donated zero) operands — results are fresh buffers.
        all_in_names = list(in_names)
        if part_name is not None:
            all_in_names.append(part_name)

        def _body(*args):
            operands = list(args)
            if part_name is not None:
                operands.append(bass2jax.partition_id_tensor())
            outs = _bass_exec_p.bind(
                *operands,
                out_avals=tuple(out_avals),
                in_names=tuple(all_in_names),
                out_names=tuple(out_names),
                lowering_input_output_aliases=(),
                sim_require_finite=True,
                sim_require_nnan=True,
                nc=nc,
            )
            return tuple(outs)

        devices = jax.devices()[:n_cores]
        self.mesh = Mesh(np.asarray(devices), ("core",))
        self.sharding = NamedSharding(self.mesh, PartitionSpec("core"))
        in_specs = (PartitionSpec("core"),) * len(in_names)
        out_specs = (PartitionSpec("core"),) * len(out_names)
        jitted = jax.jit(
            shard_map(
                _body,
                mesh=self.mesh,
                in_specs=in_specs,
                out_specs=out_specs,
                check_rep=False,
            ),
            keep_unused=True,
        )

        def compile_fn():
            abstract = [
                jax.ShapeDtypeStruct((n_cores * s[0], *s[1:]), d)
                for (s, d) in in_shapes
            ]
            return jitted.lower(*abstract).compile()

        self.compiled = fast_dispatch_compile(compile_fn)

    def stage(self, in_maps):
        """Concat per-core inputs on axis 0 and put on device (committed)."""
        concat = [
            np.concatenate(
                [np.asarray(in_maps[c][nm]) for c in range(self.n_cores)], axis=0
            )
            for nm in self.in_names
        ]
        args = [self.jax.device_put(a, self.sharding) for a in concat]
        self.jax.block_until_ready(args)
        return args

    def __call__(self, args):
        return self.compiled(*args)

    def fetch_np(self, outs):
        """outs -> list of per-core np arrays for output 0."""
        arr = np.asarray(outs[0])
        s = self.out_avals[0].shape
        return arr.reshape(self.n_cores, *s)


def _assemble(per_core_outT):
    """per-core outT [OUT_DIM, BSH] -> full [BATCH, OUT_DIM]."""
    return np.ascontiguousarray(
        np.concatenate([o.T for o in per_core_outT], axis=0), dtype=np.float32
    )


def run(inputs: dict, trace: bool = False, **spmd_kwargs):
    """Stock-path execution (kept for debugging / fallback)."""
    from concourse.bass_utils import run_bass_kernel_spmd

    in_maps, inv_h, bias_v = prepare_inputs(**inputs)
    nc = _legalize_waits(build_program(inv_h, bias_v))
    res = run_bass_kernel_spmd(
        nc, in_maps, list(range(N_CORES)), trace=trace, **spmd_kwargs
    )
    out = _assemble([np.asarray(res.results[c]["outT"]) for c in range(N_CORES)])
    return out, res


def kernel(**inputs) -> np.ndarray:
    assert inputs["x"].shape == (BATCH, IN_DIM)
    in_maps, inv_h, bias_v = prepare_inputs(**inputs)
    nc = _legalize_waits(build_program(inv_h, bias_v))
    try:
        runner = Runner(nc)
        outs = runner(runner.stage(in_maps))
        return _assemble(list(runner.fetch_np(outs)))
    except Exception:
        from concourse.bass_utils import run_bass_kernel_spmd

        res = run_bass_kernel_spmd(nc, in_maps, list(range(N_CORES)))
        return _assemble(
            [np.asarray(res.results[c]["outT"]) for c in range(N_CORES)]
        )


# revision 30
# speedup vs baseline: 1.0779x; 1.0021x over previous
"""KANLayer (in=128, out=128, num=5, k=3, batch=1024) on 8 trn2 NeuronCores.

Math: out[b,o] = sum_i mask*scale_base*silu(x[b,i])
              + sum_i mask*scale_sp*sum_j coef[(o,i),j]*B_j(x[b,i])
The reference grid is a uniform linspace broadcast to all rows, so the
Cox-de-Boor bases are cardinal cubic B-splines: B_j(x) = N3(v - j) with
v = (x - g0ext)/h.  N3 is evaluated in closed (truncated-power) form:
6*N3(v - j) = Delta^4[relu(v - n)^3] at n = j, so the whole basis bank
[128 i-lanes, 8 bases, batch] comes from one relu^3 over 12 taps plus
four shifted-slice subtracts (the 1/6 is folded into the spline weights
host-side).  The contraction over (i, j) and the silu base term are 9
accumulated 128x128x128 PE matmuls per core (bf16 in, f32 accumulate).

Valid for |x| <= g_ext_max (|x| <= 8.8 here); beyond that the Delta^4
cancellation noise grows cubically.  setup_inputs' randn never leaves
[-5, 5].

Sharding: batch 1024 -> 128 per core (independent; no collectives).

Execution: the Bass program is AOT-compiled once into a PJRT executable
(fast-dispatch, no per-call retrace) and dispatched on cores 0-7; falls
back to the stock run_bass_kernel_spmd path on any failure.
"""

import numpy as np

import concourse.bass as bass
import concourse.mybir as mybir
import concourse.tile as tile

AF = mybir.ActivationFunctionType
ALU = mybir.AluOpType
F32 = mybir.dt.float32
BF16 = mybir.dt.bfloat16

N_CORES = 8
BATCH = 1024
IN_DIM = 128
OUT_DIM = 128
NUM, KDEG = 5, 3
NB = NUM + KDEG          # 8 basis functions
NK = 1 + NB              # 9 matmul K-tiles (silu + 8 bases)
NT = NB + 4              # 12 truncated-power taps relu(v-n)^3, n = 0..11
BSH = BATCH // N_CORES   # 128 batch elems per core
SIZE = IN_DIM * OUT_DIM


def _bcast_mid(ap2d, n):
    """[128, F] AP -> [128, n, F] with zero-stride middle dim."""
    p, f = ap2d.shape
    return ap2d.rearrange("p (a b) -> p a b", a=1).broadcast_to([p, n, f])


def _flat(ap3d):
    """[128, a, b] AP -> [128, a*b]."""
    return ap3d.rearrange("p a b -> p (a b)")


MM_DT = BF16  # matmul operand dtype (weights, silu, basis bank)

# Live-tap window, set by prepare_inputs from the actual data range:
# tap n contributes only if max(v) > n, and relu is needed for tap n only
# if min(v) < n.  For randn x on this grid: v in [2.7, 8.3] -> L=9, N0=3.
TAP_L = NT
TAP_N0 = 0


def _emit_iter(nc, pool, psum, xs, wt, outT, inv_h, bias_v, split=False):
    """One full kernel pass: load, spline-basis bank, 9 matmuls, store.

    outT is a [OUT_DIM, BSH] dram AP (a per-iteration slice when unrolled
    pipelined).  split=False keeps the whole basis chain on the vector
    engine — same-engine deps are free and cross-engine hops cost ~1us,
    which is optimal when iterations serialize.  split=True interleaves
    vector and gpsimd per level for the double-buffered streaming case,
    where hops hide and the per-engine queue length is what matters.
    Every Delta level is one flat 2D op (in1 = same buffer shifted by BSH
    elements); 3D strided APs cost ~0.7us extra per op on DVE.
    """
    L = TAP_L  # L in [9, 12]
    eng2 = nc.gpsimd if split else nc.vector
    # double-buffered input loads: the next pass's DMA issues while this
    # pass computes, hiding the ~5us DMA latency (standard prefetch)
    X = pool.tile([128, BSH], F32, tag="X", bufs=2)
    nc.sync.dma_start(X[:], xs[:])
    WT = pool.tile([128, NK, OUT_DIM], MM_DT, tag="WT", bufs=2)
    nc.sync.dma_start(WT[:].rearrange("p a b -> p (a b)"), wt[:])

    S = pool.tile([128, BSH], MM_DT, tag="S")  # silu(x), matmul K-tile 0
    nc.scalar.activation(S[:], X[:], AF.Silu)

    # negated tap offsets -n (loop-constant; double-buffered so the next
    # iteration's iota never waits on this one's reader)
    ICW = pool.tile([128, L, BSH], F32, tag="ICW", bufs=2)
    nc.gpsimd.iota(
        ICW[:], pattern=[[-1, L], [0, BSH]], base=0, channel_multiplier=0,
        allow_small_or_imprecise_dtypes=True,
    )

    V = pool.tile([128, BSH], F32, tag="V")    # v = x/h - g0ext/h
    nc.vector.tensor_scalar(V[:], X[:], inv_h, bias_v, ALU.mult, ALU.add)
    D = pool.tile([128, L, BSH], F32, tag="D")      # v - n
    nc.vector.tensor_tensor(D[:], ICW[:], _bcast_mid(V[:], L), ALU.add)
    # relu on all taps (identity on the always-positive low taps)
    R = pool.tile([128, L, BSH], F32, tag="R")      # relu(v - n)
    nc.vector.tensor_scalar(_flat(R[:]), _flat(D[:]), 0.0, None, ALU.max)
    R2 = pool.tile([128, L, BSH], F32, tag="R2")
    eng2.tensor_tensor(_flat(R2[:]), _flat(R[:]), _flat(R[:]), ALU.mult)
    R3 = pool.tile([128, L + 1, BSH], F32, tag="R3")
    R3f = _flat(R3[:])
    nc.vector.tensor_tensor(R3f[:, : L * BSH], _flat(R2[:]), _flat(R[:]),
                            ALU.mult)
    eng2.memset(R3f[:, L * BSH :], 0.0)

    # BB[j] = Delta^4 R3 | j == 6 * B_j(v).  Rows >= L of each level are
    # identically zero (taps above the data range), kept as memset tail
    # rows so every level stays one op.
    D1 = pool.tile([128, L + 1, BSH], F32, tag="D1")
    D1f = _flat(D1[:])
    eng2.tensor_tensor(D1f[:, : L * BSH], R3f[:, BSH:],
                       R3f[:, : L * BSH], ALU.subtract)
    eng2.memset(D1f[:, L * BSH :], 0.0)
    D2 = pool.tile([128, L + 1, BSH], F32, tag="D2")
    D2f = _flat(D2[:])
    nc.vector.tensor_tensor(D2f[:, : L * BSH], D1f[:, BSH:],
                            D1f[:, : L * BSH], ALU.subtract)
    nc.vector.memset(D2f[:, L * BSH :], 0.0)
    D3 = pool.tile([128, NB + 1, BSH], F32, tag="D3")
    D3f = _flat(D3[:])
    eng2.tensor_tensor(D3f[:], D2f[:, BSH : (NB + 2) * BSH],
                       D2f[:, : (NB + 1) * BSH], ALU.subtract)
    BB = pool.tile([128, NB, BSH], MM_DT, tag="BB")
    nc.vector.tensor_tensor(_flat(BB[:]), D3f[:, BSH:], D3f[:, : NB * BSH],
                            ALU.subtract)

    # out^T[o,b] = sum_k WT[:,k,:]^T @ rhs_k, K = 9*128
    PS = psum.tile([OUT_DIM, BSH], F32, tag="PS", bufs=2)
    for k in range(NK):
        rhs = S[:] if k == 0 else BB[:, k - 1, :]
        nc.tensor.matmul(
            PS[:], WT[:, k, :], rhs, start=(k == 0), stop=(k == NK - 1)
        )
    # O double-buffered: the next pass's PSUM copy must not wait for this
    # pass's out-DMA (~5us latency) to release the staging tile
    O = pool.tile([OUT_DIM, BSH], F32, tag="O", bufs=2)
    nc.scalar.copy(O[:], PS[:])
    nc.sync.dma_start(outT[:, :], O[:])


def build_program(
    inv_h: float, bias_v: float, iters: int = 1, pipelined: bool = False
):
    """One SPMD NeuronCore program; per-core inputs differ only in data.

    iters > 1 unrolls the full kernel back-to-back inside one NEFF — used
    to measure per-iteration HW execution time without a profiler.
    pipelined=True double-buffers tiles and gives each iteration its own
    output slice (streaming steady state); False reuses single buffers,
    serializing iterations (per-pass latency).
    """
    nc = bass.Bass()
    xs = nc.declare_dram_parameter("xs", [IN_DIM, BSH], F32, isOutput=False)
    # weights pre-transposed host-side to [i, k*o] so the load is one
    # contiguous-per-partition DMA (the [k*i, o] layout needs a 1152-
    # descriptor gather, ~3us of DMA-queue time per pass)
    wt = nc.declare_dram_parameter(
        "wt", [128, NK * OUT_DIM], MM_DT, isOutput=False
    )
    n_out = iters if pipelined else 1
    outT = nc.declare_dram_parameter(
        "outT", [OUT_DIM, n_out * BSH], F32, isOutput=True
    )

    with tile.TileContext(nc) as tc:
        with (
            tc.tile_pool(name="pool", bufs=2 if pipelined else 1) as pool,
            tc.tile_pool(
                name="psum", bufs=2 if pipelined else 1,
                space=bass.MemorySpace.PSUM,
            ) as psum,
        ):
            for it in range(iters):
                o = outT[:, it * BSH : (it + 1) * BSH] if pipelined else outT[:]
                _emit_iter(nc, pool, psum, xs, wt, o, inv_h, bias_v,
                           split=pipelined)

    return nc


def _legalize_waits(nc):
    """Walrus codegen allows only one semaphore wait per compute/DMA
    instruction; move extra waits onto inserted same-engine NoOps."""
    for blk in nc.m.functions[0].blocks:
        out = []
        for ins in blk.instructions:
            si = ins.sync_info
            if si is not None and len(si.on_wait) > 1:
                waits = list(si.on_wait)
                for i, w in enumerate(waits[:-1]):
                    nop = mybir.InstNoOp(
                        name=f"{ins.name}-lw{i}", engine=ins.engine, ins=[], outs=[]
                    )
                    nop.sync_info = mybir.SyncInfo(on_wait=[w], on_update=[])
                    out.append(nop)
                ins.sync_info = mybir.SyncInfo(
                    on_wait=[waits[-1]], on_update=list(si.on_update)
                )
            out.append(ins)
        blk.instructions = out
    return nc


def prepare_inputs(x, grid, coef, scale_base, scale_sp, mask):
    global TAP_L, TAP_N0
    x = np.ascontiguousarray(x, dtype=np.float32)
    grid = np.asarray(grid, dtype=np.float32)
    coef = np.asarray(coef, dtype=np.float32)
    g = grid[0].astype(np.float64)
    h = (g[-1] - g[0]) / (len(g) - 1)
    g0ext = g[0] - KDEG * h
    inv_h = 1.0 / h
    bias_v = -g0ext * inv_h

    # live-tap window from the actual data range (v = x*inv_h + bias_v):
    # taps >= L are identically zero, taps < N0 never need the relu
    vmin = float(x.min()) * inv_h + bias_v
    vmax = float(x.max()) * inv_h + bias_v
    TAP_L = int(min(max(np.floor(vmax) + 1, 9), NT))
    TAP_N0 = int(max(min(np.floor(vmin) + 1, TAP_L), 0))

    import ml_dtypes

    sbm = (np.asarray(scale_base) * np.asarray(mask)).astype(np.float32)
    sspm = (np.asarray(scale_sp) * np.asarray(mask)).astype(np.float32)
    wt = np.empty((NK * 128, OUT_DIM), np.float32)
    wt[0:128] = sbm.reshape(OUT_DIM, IN_DIM).T
    # fold the 1/6 of the truncated-power form into the spline weights
    for j in range(NB):
        wt[(j + 1) * 128 : (j + 2) * 128] = (
            (sspm * coef[:, j] / 6.0).reshape(OUT_DIM, IN_DIM).T
        )
    # [k*i, o] -> [i, k*o] so each partition's weights are contiguous
    wt = np.ascontiguousarray(
        wt.reshape(NK, IN_DIM, OUT_DIM).transpose(1, 0, 2).reshape(
            IN_DIM, NK * OUT_DIM
        )
    ).astype(mybir.dt.np(MM_DT))
    xT = np.ascontiguousarray(x.T)  # [i, b]
    in_maps = [
        {
            "xs": np.ascontiguousarray(xT[:, c * BSH : (c + 1) * BSH]),
            "wt": wt,
        }
        for c in range(N_CORES)
    ]
    return in_maps, float(inv_h), float(bias_v)


class Runner:
    """AOT-compiled fast-dispatch executor for a Bass program on N cores.

    Compiles once (jit trace + NEFF build happen here, not per call);
    subsequent __call__s hit JAX's C++ fast path — per-call cost is the
    axon dispatch plus device execution only.
    """

    def __init__(self, nc, n_cores: int = N_CORES):
        import jax
        from jax.sharding import Mesh, NamedSharding, PartitionSpec

        from concourse import bass2jax
        from concourse.bass2jax import (
            _bass_exec_p,
            fast_dispatch_compile,
            install_neuronx_cc_hook,
        )

        try:
            from jax.experimental.shard_map import shard_map
        except ImportError:  # newer jax
            from jax import shard_map

        install_neuronx_cc_hook()
        self.jax = jax
        self.n_cores = n_cores
        part_name = nc.partition_id_tensor.name if nc.partition_id_tensor else None
        assert nc.dbg_addr is None

        in_names, in_shapes, out_names, out_avals = [], [], [], []
        for alloc in nc.m.functions[0].allocations:
            if not isinstance(alloc, mybir.MemoryLocationSet):
                continue
            name = alloc.memorylocations[0].name
            if alloc.kind == "ExternalInput":
                if name != part_name:
                    in_names.append(name)
                    in_shapes.append(
                        (tuple(alloc.tensor_shape), mybir.dt.np(alloc.dtype))
                    )
            elif alloc.kind == "ExternalOutput":
                out_names.append(name)
                out_avals.append(
                    jax.core.ShapedArray(
                        tuple(alloc.tensor_shape), mybir.dt.np(alloc.dtype)
                    )
                )
        self.in_names = in_names
        self.out_names = out_names
        self.out_avals = out_avals
        # The kernel writes every element of its outputs, so they are not
        # passed as (donated zero) operands — results are fresh buffers.
        all_in_names = list(in_names)
        if part_name is not None:
            all_in_names.append(part_name)

        def _body(*args):
            operands = list(args)
            if part_name is not None:
                operands.append(bass2jax.partition_id_tensor())
            outs = _bass_exec_p.bind(
                *operands,
                out_avals=tuple(out_avals),
                in_names=tuple(all_in_names),
                out_names=tuple(out_names),
                lowering_input_output_aliases=(),
                sim_require_finite=True,
                sim_require_nnan=True,
                nc=nc,
            )
            return tuple(outs)

        devices = jax.devices()[:n_cores]
        self.mesh = Mesh(np.asarray(devices), ("core",))
        self.sharding = NamedSharding(self.mesh, PartitionSpec("core"))
        in_specs = (PartitionSpec("core"),) * len(in_names)
        out_specs = (PartitionSpec("core"),) * len(out_names)
        jitted = jax.jit(
            shard_map(
                _body,
                mesh=self.mesh,
                in_specs=in_specs,
                out_specs=out_specs,
                check_rep=False,
            ),
            keep_unused=True,
        )

        def compile_fn():
            abstract = [
                jax.ShapeDtypeStruct((n_cores * s[0], *s[1:]), d)
                for (s, d) in in_shapes
            ]
            return jitted.lower(*abstract).compile()

        self.compiled = fast_dispatch_compile(compile_fn)

    def stage(self, in_maps):
        """Concat per-core inputs on axis 0 and put on device (committed)."""
        concat = [
            np.concatenate(
                [np.asarray(in_maps[c][nm]) for c in range(self.n_cores)], axis=0
            )
            for nm in self.in_names
        ]
        args = [self.jax.device_put(a, self.sharding) for a in concat]
        self.jax.block_until_ready(args)
        return args

    def __call__(self, args):
        return self.compiled(*args)

    def fetch_np(self, outs):
        """outs -> list of per-core np arrays for output 0."""
        arr = np.asarray(outs[0])
        s = self.out_avals[0].shape
        return arr.reshape(self.n_cores, *s)


def _assemble(per_core_outT):
    """per-core outT [OUT_DIM, BSH] -> full [BATCH, OUT_DIM]."""
    return np.ascontiguousarray(
        np.concatenate([o.T for o in per_core_outT], axis=0), dtype=np.float32
    )


def run(inputs: dict, trace: bool = False, **spmd_kwargs):
    """Stock-path execution (kept for debugging / fallback)."""
    from concourse.bass_utils import run_bass_kernel_spmd

    in_maps, inv_h, bias_v = prepare_inputs(**inputs)
    nc = _legalize_waits(build_program(inv_h, bias_v))
    res = run_bass_kernel_spmd(
        nc, in_maps, list(range(N_CORES)), trace=trace, **spmd_kwargs
    )
    out = _assemble([np.asarray(res.results[c]["outT"]) for c in range(N_CORES)])
    return out, res


def kernel(**inputs) -> np.ndarray:
    assert inputs["x"].shape == (BATCH, IN_DIM)
    in_maps, inv_h, bias_v = prepare_inputs(**inputs)
    nc = _legalize_waits(build_program(inv_h, bias_v))
    try:
        runner = Runner(nc)
        outs = runner(runner.stage(in_maps))
        return _assemble(list(runner.fetch_np(outs)))
    except Exception:
        from concourse.bass_utils import run_bass_kernel_spmd

        res = run_bass_kernel_spmd(nc, in_maps, list(range(N_CORES)))
        return _assemble(
            [np.asarray(res.results[c]["outT"]) for c in range(N_CORES)]
        )
